# revision 17
# baseline (speedup 1.0000x reference)
"""CRF loss kernel for Trainium2 (8 NeuronCores, data-parallel over batch).

Problem (hardcoded shapes): scores [B=128, T=256, K=64, K=64] f32,
targets [128, 256] int (flattened from_tag*K + to_tag), lengths [128] int.

loss = (sum_b fs[b, END] - gold) / B  where fs is the CRF forward
(log-domain) scan and gold is the gathered gold-path score.

Strategy (per core, 16 batch rows):
  * Linear-domain forward scan with a constant per-step 2^-7 scale that
    is folded into the scores host-side (sc' = sc - 7*ln2), so the
    device step is a pure matmul + cast:
        a_t = E'_t^T a_{t-1},   E'_t = exp(sc'_t)   (bf16)
  * Padded timesteps (t >= L_b) are overwritten host-side with an
    "identity slab" (0 on the diagonal, -100 off it), so exp() of them
    is exactly the identity matrix and the scan needs no per-row
    freezing, masking, or per-step state dumps: a_{T-1} == a_{L_b-1}
    automatically and only the final state is read back.
  * Host pre-interleaves the scores to [b][blk][kf][t_in_blk][kto]
    (bf16), so every DMA descriptor is a contiguous 2 KiB line and one
    [128, W*K] strip per batch-row pair loads with 128 descriptors.
  * Matmuls are pair-stacked bf16: lhsT = [E_2j; E_2j+1] (128x64),
    rhs = zero-staggered state columns (128x2) -> out [64, 2] in PSUM.
    The 8 pairs are split into two groups with independent PSUM/state
    tiles; group A's PSUM->SBUF stagger-copies run on DVE and group
    B's on GpSimd, so the tensor engine pipelines across groups.
  * gold: indirect DMA element-gather of the bf16 scores, masked by
    validity via a huge sentinel index (bounds check skips them), then
    a free-axis reduce; host sums the 128 partition partials.
  * Host finishes per row: fs_b = log(a_fin[END]) + L_b * 7*ln2.
"""

import math

import ml_dtypes
import numpy as np

import concourse.bacc as bacc
import concourse.bass as bass
import concourse.tile as tile
from concourse import mybir
from concourse.bass_utils import run_bass_kernel_spmd

F32 = mybir.dt.float32
BF16 = mybir.dt.bfloat16
I32 = mybir.dt.int32

B = 128
T = 256
K = 64
START = 62
END = 63
NCORES = 8
BL = B // NCORES          # 16 local batch rows per core
NPAIR = BL // 2           # 8
NGRP = 2                  # pipeline groups
PAIRS_PER_GRP = NPAIR // NGRP  # 4
W = 16                    # timesteps per strip
NBLK = T // W             # 16
G = BL * T // 128         # gold gather indices per partition (32)
LOG_C = 7.0 * math.log(2.0)  # per-step scale 2^-7, folded into scores
PAD_OFFDIAG = -100.0      # exp() == 0 in bf16
SENTINEL = 0x7FFFFF00     # OOB gather index for padded positions

BF16NP = ml_dtypes.bfloat16


def _build_nc():
    nc = bacc.Bacc("TRN2", target_bir_lowering=False)

    u = nc.dram_tensor("u", [BL, NBLK, K, W * K], BF16, kind="ExternalInput")
    sc = nc.dram_tensor("sc", [BL, T, K, K], F32, kind="ExternalInput")
    a0 = [
        nc.dram_tensor(f"a0{g}", [128, PAIRS_PER_GRP], BF16,
                       kind="ExternalInput")
        for g in range(NGRP)
    ]
    gidx = nc.dram_tensor("gidx", [128, G], I32, kind="ExternalInput")
    afin = [
        nc.dram_tensor(f"afin{g}", [128, PAIRS_PER_GRP], BF16,
                       kind="ExternalOutput")
        for g in range(NGRP)
    ]
    goldv = nc.dram_tensor("goldv", [1, 1], F32, kind="ExternalOutput")

    with tile.TileContext(nc) as tc:
        with (
            tc.tile_pool(name="strips", bufs=2) as strips,
            tc.tile_pool(name="persist", bufs=1) as persist,
            tc.tile_pool(name="pers_psum", bufs=1, space="PSUM") as pers_psum,
        ):
            # ---- gold gather (gpsimd, off the scan's critical path) ---
            idxs = persist.tile([128, G], I32, tag="idxs", name="idxs")
            gath = persist.tile([128, G], F32, tag="gath", name="gath")
            goldsb = persist.tile([1, 1], F32, tag="goldsb", name="goldsb")
            nc.gpsimd.dma_start(out=idxs[:], in_=gidx[:])
            nc.gpsimd.memset(gath[:], 0.0)
            sc_flat = sc[:].rearrange(
                "b t kf (kto one) -> (b t kf kto) one", one=1
            )
            nc.gpsimd.indirect_dma_start(
                out=gath[:],
                out_offset=None,
                in_=sc_flat,
                in_offset=bass.IndirectOffsetOnAxis(ap=idxs[:], axis=0),
                bounds_check=BL * T * K * K - 1,
                oob_is_err=False,
            )
            nc.gpsimd.tensor_reduce(
                goldsb[:], gath[:],
                axis=mybir.AxisListType.XYZWC, op=mybir.AluOpType.add,
            )
            nc.gpsimd.dma_start(out=goldv[:], in_=goldsb[:])

            # ---- persistent state tiles -------------------------------
            # a_bufs[g][r]: [128, 4] bf16, packed: col jj holds the state
            # of row 2j in partitions 0-63 and row 2j+1 in 64-127.
            a_bufs = [
                [
                    persist.tile([128, PAIRS_PER_GRP], BF16,
                                 tag=f"a{g}_{r}", name=f"a{g}_{r}")
                    for r in range(3)
                ]
                for g in range(NGRP)
            ]
            ps_bufs = [
                [
                    pers_psum.tile([128, PAIRS_PER_GRP], F32,
                                   tag=f"ps{g}_{r}", name=f"ps{g}_{r}")
                    for r in range(2)
                ]
                for g in range(NGRP)
            ]

            # init: buffer 0 <- packed a_0 from host
            for g in range(NGRP):
                nc.sync.dma_start(out=a_bufs[g][0][:], in_=a0[g][:])

            # ---- main scan --------------------------------------------
            for blk in range(NBLK):
                cur = []
                for j in range(NPAIR):
                    s = strips.tile([128, W * K], BF16, tag=f"strip{j}")
                    for h in range(2):
                        nc.sync.dma_start(
                            out=s[64 * h : 64 * h + 64, :],
                            in_=u[2 * j + h, blk],
                        )
                    nc.scalar.activation(
                        s[:], s[:], mybir.ActivationFunctionType.Exp
                    )
                    cur.append(s)

                for tl in range(W):
                    t = blk * W + tl
                    if t == 0:
                        continue
                    for g in range(NGRP):
                        ps = ps_bufs[g][t % 2]
                        a_prev = a_bufs[g][(t - 1) % 3]
                        for jj in range(PAIRS_PER_GRP):
                            j = g * PAIRS_PER_GRP + jj
                            sl = slice(tl * K, (tl + 1) * K)
                            # even row: PE tile (0, 0); odd: (64, 64)
                            nc.tensor.matmul(
                                out=ps[0:64, jj : jj + 1],
                                lhsT=cur[j][0:64, sl],
                                rhs=a_prev[0:64, jj : jj + 1],
                                start=True,
                                stop=True,
                            )
                            nc.tensor.matmul(
                                out=ps[64:128, jj : jj + 1],
                                lhsT=cur[j][64:128, sl],
                                rhs=a_prev[64:128, jj : jj + 1],
                                start=True,
                                stop=True,
                            )
                    for g in range(NGRP):
                        nc.vector.tensor_copy(
                            a_bufs[g][t % 3][:], ps_bufs[g][t % 2][:]
                        )

            # ---- final state readout ----------------------------------
            for g in range(NGRP):
                nc.sync.dma_start(
                    out=afin[g][:], in_=a_bufs[g][(T - 1) % 3][:]
                )

    return nc


_NC_CACHE = None


def _get_nc():
    global _NC_CACHE
    if _NC_CACHE is None:
        _NC_CACHE = _build_nc()
        _NC_CACHE.finalize()
    return _NC_CACHE


def _make_in_maps(scores, targets, lengths):
    scores = np.asarray(scores, dtype=np.float32)
    targets = np.asarray(targets).astype(np.int64)
    lengths = np.asarray(lengths).astype(np.int64)

    # fold the per-step 2^-7 scale into the scores, then overwrite the
    # padded timesteps with the identity slab (exp == I exactly).
    shifted = scores - np.float32(LOG_C)
    pad_slab = np.full((K, K), PAD_OFFDIAG, dtype=np.float32)
    np.fill_diagonal(pad_slab, 0.0)
    for b in range(B):
        L = int(lengths[b])
        if L < T:
            shifted[b, L:] = pad_slab

    # a_0 = exp(sc'[b, 0, START, :]) per row (includes one 2^-7 factor)
    a0_all = np.exp(shifted[:, 0, START, :])  # [B, K] f64->f32
    a0_all = a0_all.astype(np.float32)

    in_maps = []
    for c in range(NCORES):
        sl = slice(c * BL, (c + 1) * BL)
        sh = shifted[sl]          # [BL, T, K, K]
        tg = targets[sl]          # [BL, T]
        ln = lengths[sl]          # [BL]

        # interleaved layout [b][blk][kf][tin][kto], bf16
        u = np.ascontiguousarray(
            sh.reshape(BL, NBLK, W, K, K).transpose(0, 1, 3, 2, 4)
        ).astype(BF16NP).reshape(BL, NBLK, K, W * K)

        # packed a0 per group: col jj = [row 2j state; row 2j+1 state]
        a0g = []
        for g in range(NGRP):
            arr = np.zeros((128, PAIRS_PER_GRP), dtype=BF16NP)
            for jj in range(PAIRS_PER_GRP):
                for h in range(2):
                    bl = 2 * (g * PAIRS_PER_GRP + jj) + h
                    arr[64 * h : 64 * h + 64, jj] = a0_all[
                        c * BL + bl
                    ].astype(BF16NP)
            a0g.append(arr)

        # gold gather element indices into the raw f32 scores shard
        b_idx = np.arange(BL)[:, None]
        t_idx = np.arange(T)[None, :]
        flat = (b_idx * T + t_idx) * (K * K) + tg  # [BL, T]
        valid = t_idx < ln[:, None]
        flat = np.where(valid, flat, np.int64(SENTINEL))
        gidx = flat.reshape(128, G).astype(np.int32)

        in_maps.append(
            {
                "u": u,
                "sc": np.ascontiguousarray(scores[sl]),
                "a00": a0g[0],
                "a01": a0g[1],
                "gidx": np.ascontiguousarray(gidx),
            }
        )
    return in_maps, lengths


def _combine(results, lengths):
    # a_fin carries L factors of 2^-7 (one from a_0, one per real step),
    # so fs_b = log(a_fin[END]) + L_b * LOG_C; gold is gathered from the
    # raw (unshifted) scores.
    all_scores = 0.0
    gold_total = 0.0
    for c in range(NCORES):
        gold_total += float(results[c]["goldv"][0, 0])
        afin = [results[c][f"afin{g}"].astype(np.float32) for g in range(NGRP)]
        for bl in range(BL):
            L = int(lengths[c * BL + bl])
            g = bl // (2 * PAIRS_PER_GRP)
            jj = (bl // 2) % PAIRS_PER_GRP
            h = bl % 2
            a_end = float(afin[g][64 * h + END, jj])
            all_scores += math.log(a_end) + L * LOG_C
    return np.float32((all_scores - gold_total) / B)


def kernel(scores, targets, lengths, trace=False):
    nc = _get_nc()
    in_maps, ln = _make_in_maps(scores, targets, lengths)
    res = run_bass_kernel_spmd(
        nc, in_maps, core_ids=list(range(NCORES)), trace=trace
    )
    out = _combine(res.results, ln)
    if trace:
        return out, res
    return out


# revision 19
# speedup vs baseline: 1.0020x; 1.0020x over previous
"""CRF loss kernel for Trainium2 (8 NeuronCores, data-parallel over batch).

Problem (hardcoded shapes): scores [B=128, T=256, K=64, K=64] f32,
targets [128, 256] int (flattened from_tag*K + to_tag), lengths [128] int.

loss = (sum_b fs[b, END] - gold) / B  where fs is the CRF forward
(log-domain) scan and gold is the gathered gold-path score.

Strategy (per core, 16 batch rows):
  * Linear-domain forward scan with a constant per-step 2^-7 scale that
    is folded into the scores host-side (sc' = sc - 7*ln2), so the
    device step is a pure matmul + cast:
        a_t = E'_t^T a_{t-1},   E'_t = exp(sc'_t)   (bf16)
  * Padded timesteps (t >= L_b) are overwritten host-side with an
    "identity slab" (0 on the diagonal, -100 off it), so exp() of them
    is exactly the identity matrix and the scan needs no per-row
    freezing, masking, or per-step state dumps: a_{T-1} == a_{L_b-1}
    automatically and only the final state is read back.
  * Host pre-interleaves the scores to [b][blk][kf][t_in_blk][kto]
    (bf16), so every DMA descriptor is a contiguous 2 KiB line and one
    [128, W*K] strip per batch-row pair loads with 128 descriptors.
  * Matmuls are pair-stacked bf16: lhsT = [E_2j; E_2j+1] (128x64),
    rhs = zero-staggered state columns (128x2) -> out [64, 2] in PSUM.
    The 8 pairs are split into two groups with independent PSUM/state
    tiles; group A's PSUM->SBUF stagger-copies run on DVE and group
    B's on GpSimd, so the tensor engine pipelines across groups.
  * gold: indirect DMA element-gather of the bf16 scores, masked by
    validity via a huge sentinel index (bounds check skips them), then
    a free-axis reduce; host sums the 128 partition partials.
  * Host finishes per row: fs_b = log(a_fin[END]) + L_b * 7*ln2.
"""

import math

import ml_dtypes
import numpy as np

import concourse.bacc as bacc
import concourse.bass as bass
import concourse.tile as tile
from concourse import mybir
from concourse.bass_utils import run_bass_kernel_spmd

F32 = mybir.dt.float32
BF16 = mybir.dt.bfloat16
I32 = mybir.dt.int32

B = 128
T = 256
K = 64
START = 62
END = 63
NCORES = 8
BL = B // NCORES          # 16 local batch rows per core
NPAIR = BL // 2           # 8
NGRP = 2                  # pipeline groups
PAIRS_PER_GRP = NPAIR // NGRP  # 4
W = 16                    # timesteps per strip
NBLK = T // W             # 16
G = BL * T // 128         # gold gather indices per partition (32)
LOG_C = 7.0 * math.log(2.0)  # per-step scale 2^-7, folded into scores
PAD_OFFDIAG = -100.0      # exp() == 0 in bf16
SENTINEL = 0x7FFFFF00     # OOB gather index for padded positions

BF16NP = ml_dtypes.bfloat16


def _build_nc():
    nc = bacc.Bacc("TRN2", target_bir_lowering=False)

    u = nc.dram_tensor("u", [BL, NBLK, K, W * K], BF16, kind="ExternalInput")
    sc = nc.dram_tensor("sc", [BL, T, K, K], F32, kind="ExternalInput")
    a0 = [
        nc.dram_tensor(f"a0{g}", [128, PAIRS_PER_GRP], BF16,
                       kind="ExternalInput")
        for g in range(NGRP)
    ]
    gidx = nc.dram_tensor("gidx", [128, G], I32, kind="ExternalInput")
    afin = [
        nc.dram_tensor(f"afin{g}", [128, PAIRS_PER_GRP], BF16,
                       kind="ExternalOutput")
        for g in range(NGRP)
    ]
    goldv = nc.dram_tensor("goldv", [1, 1], F32, kind="ExternalOutput")

    with tile.TileContext(nc) as tc:
        with (
            tc.tile_pool(name="strips", bufs=2) as strips,
            tc.tile_pool(name="persist", bufs=1) as persist,
            tc.tile_pool(name="pers_psum", bufs=1, space="PSUM") as pers_psum,
        ):
            # ---- gold gather (gpsimd, off the scan's critical path) ---
            idxs = persist.tile([128, G], I32, tag="idxs", name="idxs")
            gath = persist.tile([128, G], F32, tag="gath", name="gath")
            goldsb = persist.tile([1, 1], F32, tag="goldsb", name="goldsb")
            nc.gpsimd.dma_start(out=idxs[:], in_=gidx[:])
            nc.gpsimd.memset(gath[:], 0.0)
            sc_flat = sc[:].rearrange(
                "b t kf (kto one) -> (b t kf kto) one", one=1
            )
            nc.gpsimd.indirect_dma_start(
                out=gath[:],
                out_offset=None,
                in_=sc_flat,
                in_offset=bass.IndirectOffsetOnAxis(ap=idxs[:], axis=0),
                bounds_check=BL * T * K * K - 1,
                oob_is_err=False,
            )
            nc.gpsimd.tensor_reduce(
                goldsb[:], gath[:],
                axis=mybir.AxisListType.XYZWC, op=mybir.AluOpType.add,
            )
            nc.gpsimd.dma_start(out=goldv[:], in_=goldsb[:])

            # ---- persistent state tiles -------------------------------
            # a_bufs[g][r]: [128, 4] bf16, packed: col jj holds the state
            # of row 2j in partitions 0-63 and row 2j+1 in 64-127.
            a_bufs = [
                [
                    persist.tile([128, PAIRS_PER_GRP], BF16,
                                 tag=f"a{g}_{r}", name=f"a{g}_{r}")
                    for r in range(3)
                ]
                for g in range(NGRP)
            ]
            # each PSUM tile padded to a full 2KB bank so the four
            # rotating tiles land in distinct banks (a shared bank
            # serializes group A's copy against group B's matmuls).
            ps_bufs = [
                [
                    pers_psum.tile([128, 512], F32,
                                   tag=f"ps{g}_{r}", name=f"ps{g}_{r}")
                    for r in range(2)
                ]
                for g in range(NGRP)
            ]

            # init: buffer 0 <- packed a_0 from host
            for g in range(NGRP):
                nc.sync.dma_start(out=a_bufs[g][0][:], in_=a0[g][:])

            # ---- main scan --------------------------------------------
            for blk in range(NBLK):
                cur = []
                for j in range(NPAIR):
                    s = strips.tile([128, W * K], BF16, tag=f"strip{j}")
                    for h in range(2):
                        nc.sync.dma_start(
                            out=s[64 * h : 64 * h + 64, :],
                            in_=u[2 * j + h, blk],
                        )
                    nc.scalar.activation(
                        s[:], s[:], mybir.ActivationFunctionType.Exp
                    )
                    cur.append(s)

                for tl in range(W):
                    t = blk * W + tl
                    if t == 0:
                        continue
                    for g in range(NGRP):
                        ps = ps_bufs[g][t % 2]
                        a_prev = a_bufs[g][(t - 1) % 3]
                        for jj in range(PAIRS_PER_GRP):
                            j = g * PAIRS_PER_GRP + jj
                            sl = slice(tl * K, (tl + 1) * K)
                            # even row: PE tile (0, 0); odd: (64, 64)
                            nc.tensor.matmul(
                                out=ps[0:64, jj : jj + 1],
                                lhsT=cur[j][0:64, sl],
                                rhs=a_prev[0:64, jj : jj + 1],
                                start=True,
                                stop=True,
                            )
                            nc.tensor.matmul(
                                out=ps[64:128, jj : jj + 1],
                                lhsT=cur[j][64:128, sl],
                                rhs=a_prev[64:128, jj : jj + 1],
                                start=True,
                                stop=True,
                            )
                    for g in range(NGRP):
                        nc.vector.tensor_copy(
                            a_bufs[g][t % 3][:],
                            ps_bufs[g][t % 2][:, 0:PAIRS_PER_GRP],
                        )

            # ---- final state readout ----------------------------------
            for g in range(NGRP):
                nc.sync.dma_start(
                    out=afin[g][:], in_=a_bufs[g][(T - 1) % 3][:]
                )

    return nc


_NC_CACHE = None


def _get_nc():
    global _NC_CACHE
    if _NC_CACHE is None:
        _NC_CACHE = _build_nc()
        _NC_CACHE.finalize()
    return _NC_CACHE


def _make_in_maps(scores, targets, lengths):
    scores = np.asarray(scores, dtype=np.float32)
    targets = np.asarray(targets).astype(np.int64)
    lengths = np.asarray(lengths).astype(np.int64)

    # fold the per-step 2^-7 scale into the scores, then overwrite the
    # padded timesteps with the identity slab (exp == I exactly).
    shifted = scores - np.float32(LOG_C)
    pad_slab = np.full((K, K), PAD_OFFDIAG, dtype=np.float32)
    np.fill_diagonal(pad_slab, 0.0)
    for b in range(B):
        L = int(lengths[b])
        if L < T:
            shifted[b, L:] = pad_slab

    # a_0 = exp(sc'[b, 0, START, :]) per row (includes one 2^-7 factor)
    a0_all = np.exp(shifted[:, 0, START, :])  # [B, K] f64->f32
    a0_all = a0_all.astype(np.float32)

    in_maps = []
    for c in range(NCORES):
        sl = slice(c * BL, (c + 1) * BL)
        sh = shifted[sl]          # [BL, T, K, K]
        tg = targets[sl]          # [BL, T]
        ln = lengths[sl]          # [BL]

        # interleaved layout [b][blk][kf][tin][kto], bf16
        u = np.ascontiguousarray(
            sh.reshape(BL, NBLK, W, K, K).transpose(0, 1, 3, 2, 4)
        ).astype(BF16NP).reshape(BL, NBLK, K, W * K)

        # packed a0 per group: col jj = [row 2j state; row 2j+1 state]
        a0g = []
        for g in range(NGRP):
            arr = np.zeros((128, PAIRS_PER_GRP), dtype=BF16NP)
            for jj in range(PAIRS_PER_GRP):
                for h in range(2):
                    bl = 2 * (g * PAIRS_PER_GRP + jj) + h
                    arr[64 * h : 64 * h + 64, jj] = a0_all[
                        c * BL + bl
                    ].astype(BF16NP)
            a0g.append(arr)

        # gold gather element indices into the raw f32 scores shard
        b_idx = np.arange(BL)[:, None]
        t_idx = np.arange(T)[None, :]
        flat = (b_idx * T + t_idx) * (K * K) + tg  # [BL, T]
        valid = t_idx < ln[:, None]
        flat = np.where(valid, flat, np.int64(SENTINEL))
        gidx = flat.reshape(128, G).astype(np.int32)

        in_maps.append(
            {
                "u": u,
                "sc": np.ascontiguousarray(scores[sl]),
                "a00": a0g[0],
                "a01": a0g[1],
                "gidx": np.ascontiguousarray(gidx),
            }
        )
    return in_maps, lengths


def _combine(results, lengths):
    # a_fin carries L factors of 2^-7 (one from a_0, one per real step),
    # so fs_b = log(a_fin[END]) + L_b * LOG_C; gold is gathered from the
    # raw (unshifted) scores.
    all_scores = 0.0
    gold_total = 0.0
    for c in range(NCORES):
        gold_total += float(results[c]["goldv"][0, 0])
        afin = [results[c][f"afin{g}"].astype(np.float32) for g in range(NGRP)]
        for bl in range(BL):
            L = int(lengths[c * BL + bl])
            g = bl // (2 * PAIRS_PER_GRP)
            jj = (bl // 2) % PAIRS_PER_GRP
            h = bl % 2
            a_end = float(afin[g][64 * h + END, jj])
            all_scores += math.log(a_end) + L * LOG_C
    return np.float32((all_scores - gold_total) / B)


def kernel(scores, targets, lengths, trace=False):
    nc = _get_nc()
    in_maps, ln = _make_in_maps(scores, targets, lengths)
    res = run_bass_kernel_spmd(
        nc, in_maps, core_ids=list(range(NCORES)), trace=trace
    )
    out = _combine(res.results, ln)
    if trace:
        return out, res
    return out


# revision 20
# speedup vs baseline: 1.0042x; 1.0022x over previous
"""CRF loss kernel for Trainium2 (8 NeuronCores, data-parallel over batch).

Problem (hardcoded shapes): scores [B=128, T=256, K=64, K=64] f32,
targets [128, 256] int (flattened from_tag*K + to_tag), lengths [128] int.

loss = (sum_b fs[b, END] - gold) / B  where fs is the CRF forward
(log-domain) scan and gold is the gathered gold-path score.

Strategy (per core, 16 batch rows):
  * Linear-domain forward scan with a constant per-step 2^-7 scale that
    is folded into the scores host-side (sc' = sc - 7*ln2), so the
    device step is a pure matmul + cast:
        a_t = E'_t^T a_{t-1},   E'_t = exp(sc'_t)   (bf16)
  * Padded timesteps (t >= L_b) are overwritten host-side with an
    "identity slab" (0 on the diagonal, -100 off it), so exp() of them
    is exactly the identity matrix and the scan needs no per-row
    freezing, masking, or per-step state dumps: a_{T-1} == a_{L_b-1}
    automatically and only the final state is read back.
  * Host pre-interleaves the scores to [b][blk][kf][t_in_blk][kto]
    (bf16), so every DMA descriptor is a contiguous 2 KiB line and one
    [128, W*K] strip per batch-row pair loads with 128 descriptors.
  * Matmuls are pair-stacked bf16: lhsT = [E_2j; E_2j+1] (128x64),
    rhs = zero-staggered state columns (128x2) -> out [64, 2] in PSUM.
    The 8 pairs are split into two groups with independent PSUM/state
    tiles; group A's PSUM->SBUF stagger-copies run on DVE and group
    B's on GpSimd, so the tensor engine pipelines across groups.
  * gold: indirect DMA element-gather of the bf16 scores, masked by
    validity via a huge sentinel index (bounds check skips them), then
    a free-axis reduce; host sums the 128 partition partials.
  * Host finishes per row: fs_b = log(a_fin[END]) + L_b * 7*ln2.
"""

import math

import ml_dtypes
import numpy as np

import concourse.bacc as bacc
import concourse.bass as bass
import concourse.tile as tile
from concourse import mybir
from concourse.bass_utils import run_bass_kernel_spmd

F32 = mybir.dt.float32
BF16 = mybir.dt.bfloat16
I32 = mybir.dt.int32

B = 128
T = 256
K = 64
START = 62
END = 63
NCORES = 8
BL = B // NCORES          # 16 local batch rows per core
NPAIR = BL // 2           # 8
NGRP = 2                  # pipeline groups
PAIRS_PER_GRP = NPAIR // NGRP  # 4
W = 16                    # timesteps per strip
NBLK = T // W             # 16
G = BL * T // 128         # gold gather indices per partition (32)
LOG_C = 7.0 * math.log(2.0)  # per-step scale 2^-7, folded into scores
PAD_OFFDIAG = -100.0      # exp() == 0 in bf16
SENTINEL = 0x7FFFFF00     # OOB gather index for padded positions

BF16NP = ml_dtypes.bfloat16


def _build_nc():
    nc = bacc.Bacc("TRN2", target_bir_lowering=False)

    u = nc.dram_tensor("u", [BL, NBLK, K, W * K], BF16, kind="ExternalInput")
    sc = nc.dram_tensor("sc", [BL, T, K, K], F32, kind="ExternalInput")
    a0 = [
        nc.dram_tensor(f"a0{g}", [128, PAIRS_PER_GRP], BF16,
                       kind="ExternalInput")
        for g in range(NGRP)
    ]
    gidx = nc.dram_tensor("gidx", [128, G], I32, kind="ExternalInput")
    afin = [
        nc.dram_tensor(f"afin{g}", [128, PAIRS_PER_GRP], BF16,
                       kind="ExternalOutput")
        for g in range(NGRP)
    ]
    goldv = nc.dram_tensor("goldv", [1, 1], F32, kind="ExternalOutput")

    with tile.TileContext(nc) as tc:
        with (
            tc.tile_pool(name="strips", bufs=2) as strips,
            tc.tile_pool(name="persist", bufs=1) as persist,
            tc.tile_pool(name="pers_psum", bufs=1, space="PSUM") as pers_psum,
        ):
            # ---- gold gather (gpsimd, off the scan's critical path) ---
            idxs = persist.tile([128, G], I32, tag="idxs", name="idxs")
            gath = persist.tile([128, G], F32, tag="gath", name="gath")
            goldsb = persist.tile([1, 1], F32, tag="goldsb", name="goldsb")
            nc.gpsimd.dma_start(out=idxs[:], in_=gidx[:])
            nc.gpsimd.memset(gath[:], 0.0)
            sc_flat = sc[:].rearrange(
                "b t kf (kto one) -> (b t kf kto) one", one=1
            )
            nc.gpsimd.indirect_dma_start(
                out=gath[:],
                out_offset=None,
                in_=sc_flat,
                in_offset=bass.IndirectOffsetOnAxis(ap=idxs[:], axis=0),
                bounds_check=BL * T * K * K - 1,
                oob_is_err=False,
            )
            nc.gpsimd.tensor_reduce(
                goldsb[:], gath[:],
                axis=mybir.AxisListType.XYZWC, op=mybir.AluOpType.add,
            )
            nc.gpsimd.dma_start(out=goldv[:], in_=goldsb[:])

            # ---- persistent state tiles -------------------------------
            # a_bufs[g][r]: [128, 4] bf16, packed: col jj holds the state
            # of row 2j in partitions 0-63 and row 2j+1 in 64-127.
            a_bufs = [
                [
                    persist.tile([128, PAIRS_PER_GRP], BF16,
                                 tag=f"a{g}_{r}", name=f"a{g}_{r}")
                    for r in range(3)
                ]
                for g in range(NGRP)
            ]
            # each PSUM tile padded to a full 2KB bank so the four
            # rotating tiles land in distinct banks (a shared bank
            # serializes group A's copy against group B's matmuls).
            ps_bufs = [
                [
                    pers_psum.tile([128, 512], F32,
                                   tag=f"ps{g}_{r}", name=f"ps{g}_{r}")
                    for r in range(2)
                ]
                for g in range(NGRP)
            ]

            # init: buffer 0 <- packed a_0 from host
            for g in range(NGRP):
                nc.sync.dma_start(out=a_bufs[g][0][:], in_=a0[g][:])

            # ---- main scan --------------------------------------------
            for blk in range(NBLK):
                cur = []
                for j in range(NPAIR):
                    s = strips.tile([128, W * K], BF16, tag=f"strip{j}")
                    for h in range(2):
                        nc.sync.dma_start(
                            out=s[64 * h : 64 * h + 64, :],
                            in_=u[2 * j + h, blk],
                        )
                    nc.scalar.activation(
                        s[:], s[:], mybir.ActivationFunctionType.Exp
                    )
                    cur.append(s)

                for tl in range(W):
                    t = blk * W + tl
                    if t == 0:
                        continue
                    for g in range(NGRP):
                        ps = ps_bufs[g][t % 2]
                        a_prev = a_bufs[g][(t - 1) % 3]
                        for jj in range(PAIRS_PER_GRP):
                            j = g * PAIRS_PER_GRP + jj
                            sl = slice(tl * K, (tl + 1) * K)
                            # even row: PE tile (0, 0); odd: (64, 64)
                            nc.tensor.matmul(
                                out=ps[0:64, jj : jj + 1],
                                lhsT=cur[j][0:64, sl],
                                rhs=a_prev[0:64, jj : jj + 1],
                                start=True,
                                stop=True,
                            )
                            nc.tensor.matmul(
                                out=ps[64:128, jj : jj + 1],
                                lhsT=cur[j][64:128, sl],
                                rhs=a_prev[64:128, jj : jj + 1],
                                start=True,
                                stop=True,
                            )
                        nc.vector.tensor_copy(
                            a_bufs[g][t % 3][:],
                            ps[:, 0:PAIRS_PER_GRP],
                        )

            # ---- final state readout ----------------------------------
            for g in range(NGRP):
                nc.sync.dma_start(
                    out=afin[g][:], in_=a_bufs[g][(T - 1) % 3][:]
                )

    return nc


_NC_CACHE = None


def _get_nc():
    global _NC_CACHE
    if _NC_CACHE is None:
        _NC_CACHE = _build_nc()
        _NC_CACHE.finalize()
    return _NC_CACHE


def _make_in_maps(scores, targets, lengths):
    scores = np.asarray(scores, dtype=np.float32)
    targets = np.asarray(targets).astype(np.int64)
    lengths = np.asarray(lengths).astype(np.int64)

    # fold the per-step 2^-7 scale into the scores, then overwrite the
    # padded timesteps with the identity slab (exp == I exactly).
    shifted = scores - np.float32(LOG_C)
    pad_slab = np.full((K, K), PAD_OFFDIAG, dtype=np.float32)
    np.fill_diagonal(pad_slab, 0.0)
    for b in range(B):
        L = int(lengths[b])
        if L < T:
            shifted[b, L:] = pad_slab

    # a_0 = exp(sc'[b, 0, START, :]) per row (includes one 2^-7 factor)
    a0_all = np.exp(shifted[:, 0, START, :])  # [B, K] f64->f32
    a0_all = a0_all.astype(np.float32)

    in_maps = []
    for c in range(NCORES):
        sl = slice(c * BL, (c + 1) * BL)
        sh = shifted[sl]          # [BL, T, K, K]
        tg = targets[sl]          # [BL, T]
        ln = lengths[sl]          # [BL]

        # interleaved layout [b][blk][kf][tin][kto], bf16
        u = np.ascontiguousarray(
            sh.reshape(BL, NBLK, W, K, K).transpose(0, 1, 3, 2, 4)
        ).astype(BF16NP).reshape(BL, NBLK, K, W * K)

        # packed a0 per group: col jj = [row 2j state; row 2j+1 state]
        a0g = []
        for g in range(NGRP):
            arr = np.zeros((128, PAIRS_PER_GRP), dtype=BF16NP)
            for jj in range(PAIRS_PER_GRP):
                for h in range(2):
                    bl = 2 * (g * PAIRS_PER_GRP + jj) + h
                    arr[64 * h : 64 * h + 64, jj] = a0_all[
                        c * BL + bl
                    ].astype(BF16NP)
            a0g.append(arr)

        # gold gather element indices into the raw f32 scores shard
        b_idx = np.arange(BL)[:, None]
        t_idx = np.arange(T)[None, :]
        flat = (b_idx * T + t_idx) * (K * K) + tg  # [BL, T]
        valid = t_idx < ln[:, None]
        flat = np.where(valid, flat, np.int64(SENTINEL))
        gidx = flat.reshape(128, G).astype(np.int32)

        in_maps.append(
            {
                "u": u,
                "sc": np.ascontiguousarray(scores[sl]),
                "a00": a0g[0],
                "a01": a0g[1],
                "gidx": np.ascontiguousarray(gidx),
            }
        )
    return in_maps, lengths


def _combine(results, lengths):
    # a_fin carries L factors of 2^-7 (one from a_0, one per real step),
    # so fs_b = log(a_fin[END]) + L_b * LOG_C; gold is gathered from the
    # raw (unshifted) scores.
    all_scores = 0.0
    gold_total = 0.0
    for c in range(NCORES):
        gold_total += float(results[c]["goldv"][0, 0])
        afin = [results[c][f"afin{g}"].astype(np.float32) for g in range(NGRP)]
        for bl in range(BL):
            L = int(lengths[c * BL + bl])
            g = bl // (2 * PAIRS_PER_GRP)
            jj = (bl // 2) % PAIRS_PER_GRP
            h = bl % 2
            a_end = float(afin[g][64 * h + END, jj])
            all_scores += math.log(a_end) + L * LOG_C
    return np.float32((all_scores - gold_total) / B)


def kernel(scores, targets, lengths, trace=False):
    nc = _get_nc()
    in_maps, ln = _make_in_maps(scores, targets, lengths)
    res = run_bass_kernel_spmd(
        nc, in_maps, core_ids=list(range(NCORES)), trace=trace
    )
    out = _combine(res.results, ln)
    if trace:
        return out, res
    return out


# revision 21
# speedup vs baseline: 1.3797x; 1.3739x over previous
"""CRF loss kernel for Trainium2 (8 NeuronCores, data-parallel over batch).

Problem (hardcoded shapes): scores [B=128, T=256, K=64, K=64] f32,
targets [128, 256] int (flattened from_tag*K + to_tag), lengths [128] int.

loss = (sum_b fs[b, END] - gold) / B  where fs is the CRF forward
(log-domain) scan and gold is the gathered gold-path score.

Strategy (per core, 16 batch rows):
  * Linear-domain forward scan with a constant per-step 2^-7 scale that
    is folded into the scores host-side (sc' = sc - 7*ln2), so the
    device step is a pure matmul + cast:
        a_t = E'_t^T a_{t-1},   E'_t = exp(sc'_t)   (bf16)
  * Padded timesteps (t >= L_b) are overwritten host-side with an
    "identity slab" (0 on the diagonal, -100 off it), so exp() of them
    is exactly the identity matrix and the scan needs no per-row
    freezing, masking, or per-step state dumps: a_{T-1} == a_{L_b-1}
    automatically and only the final state is read back.
  * Host pre-interleaves the scores to [b][blk][kf][t_in_blk][kto]
    (bf16), so every DMA descriptor is a contiguous 2 KiB line and one
    [128, W*K] strip per batch-row pair loads with 128 descriptors.
  * Matmuls are pair-stacked bf16: lhsT = [E_2j; E_2j+1] (128x64),
    rhs = zero-staggered state columns (128x2) -> out [64, 2] in PSUM.
    The 8 pairs are split into two groups with independent PSUM/state
    tiles; group A's PSUM->SBUF stagger-copies run on DVE and group
    B's on GpSimd, so the tensor engine pipelines across groups.
  * gold: indirect DMA element-gather of the bf16 scores, masked by
    validity via a huge sentinel index (bounds check skips them), then
    a free-axis reduce; host sums the 128 partition partials.
  * Host finishes per row: fs_b = log(a_fin[END]) + L_b * 7*ln2.
"""

import math

import ml_dtypes
import numpy as np

import concourse.bacc as bacc
import concourse.bass as bass
import concourse.tile as tile
from concourse import mybir
from concourse.bass_utils import run_bass_kernel_spmd

F32 = mybir.dt.float32
BF16 = mybir.dt.bfloat16
I32 = mybir.dt.int32

B = 128
T = 256
K = 64
START = 62
END = 63
NCORES = 8
BL = B // NCORES          # 16 local batch rows per core
NPAIR = BL // 2           # 8
NGRP = 2                  # pipeline groups
PAIRS_PER_GRP = NPAIR // NGRP  # 4
W = 16                    # timesteps per strip
NBLK = T // W             # 16
G = BL * T // 128         # gold gather indices per partition (32)
LOG_C = 7.0 * math.log(2.0)  # per-step scale 2^-7, folded into scores
PAD_OFFDIAG = -100.0      # exp() == 0 in bf16
SENTINEL = 0x7FFFFF00     # OOB gather index for padded positions

BF16NP = ml_dtypes.bfloat16


def _build_nc():
    nc = bacc.Bacc("TRN2", target_bir_lowering=False)

    u = nc.dram_tensor("u", [BL, NBLK, K, W * K], BF16, kind="ExternalInput")
    sc = nc.dram_tensor("sc", [BL, T, K, K], F32, kind="ExternalInput")
    a0 = [
        nc.dram_tensor(f"a0{g}", [128, PAIRS_PER_GRP], BF16,
                       kind="ExternalInput")
        for g in range(NGRP)
    ]
    gidx = nc.dram_tensor("gidx", [128, G], I32, kind="ExternalInput")
    afin = [
        nc.dram_tensor(f"afin{g}", [128, PAIRS_PER_GRP], BF16,
                       kind="ExternalOutput")
        for g in range(NGRP)
    ]
    goldv = nc.dram_tensor("goldv", [1, 1], F32, kind="ExternalOutput")

    with tile.TileContext(nc) as tc:
        with (
            tc.tile_pool(name="strips", bufs=2) as strips,
            tc.tile_pool(name="persist", bufs=1) as persist,
            tc.tile_pool(name="pers_psum", bufs=1, space="PSUM") as pers_psum,
        ):
            # ---- gold gather (gpsimd, off the scan's critical path) ---
            idxs = persist.tile([128, G], I32, tag="idxs", name="idxs")
            gath = persist.tile([128, G], F32, tag="gath", name="gath")
            goldsb = persist.tile([1, 1], F32, tag="goldsb", name="goldsb")
            nc.gpsimd.dma_start(out=idxs[:], in_=gidx[:])
            nc.gpsimd.memset(gath[:], 0.0)
            sc_flat = sc[:].rearrange(
                "b t kf (kto one) -> (b t kf kto) one", one=1
            )
            nc.gpsimd.indirect_dma_start(
                out=gath[:],
                out_offset=None,
                in_=sc_flat,
                in_offset=bass.IndirectOffsetOnAxis(ap=idxs[:], axis=0),
                bounds_check=BL * T * K * K - 1,
                oob_is_err=False,
            )
            nc.gpsimd.tensor_reduce(
                goldsb[:], gath[:],
                axis=mybir.AxisListType.XYZWC, op=mybir.AluOpType.add,
            )
            nc.gpsimd.dma_start(out=goldv[:], in_=goldsb[:])

            # ---- persistent state tiles -------------------------------
            # a_bufs[g][r]: [128, 4] bf16, packed: col jj holds the state
            # of row 2j in partitions 0-63 and row 2j+1 in 64-127.
            a_bufs = [
                [
                    persist.tile([128, PAIRS_PER_GRP], BF16,
                                 tag=f"a{g}_{r}", name=f"a{g}_{r}")
                    for r in range(3)
                ]
                for g in range(NGRP)
            ]
            # each PSUM tile padded to a full 2KB bank so the four
            # rotating tiles land in distinct banks (a shared bank
            # serializes group A's copy against group B's matmuls).
            ps_bufs = [
                [
                    pers_psum.tile([128, 512], F32,
                                   tag=f"ps{g}_{r}", name=f"ps{g}_{r}")
                    for r in range(2)
                ]
                for g in range(NGRP)
            ]

            # init: buffer 0 <- packed a_0 from host
            for g in range(NGRP):
                nc.sync.dma_start(out=a_bufs[g][0][:], in_=a0[g][:])

            # ---- main scan --------------------------------------------
            for blk in range(NBLK):
                # alternate groups in strip emission order so neither
                # group's exp()s systematically finish first and push the
                # scheduler into serializing the groups.
                cur = [None] * NPAIR
                for j in (0, 4, 1, 5, 2, 6, 3, 7):
                    s = strips.tile([128, W * K], BF16, tag=f"strip{j}")
                    for h in range(2):
                        nc.sync.dma_start(
                            out=s[64 * h : 64 * h + 64, :],
                            in_=u[2 * j + h, blk],
                        )
                    nc.scalar.activation(
                        s[:], s[:], mybir.ActivationFunctionType.Exp
                    )
                    cur[j] = s

                for tl in range(W):
                    t = blk * W + tl
                    if t == 0:
                        continue
                    for g in range(NGRP):
                        ps = ps_bufs[g][t % 2]
                        a_prev = a_bufs[g][(t - 1) % 3]
                        for jj in range(PAIRS_PER_GRP):
                            j = g * PAIRS_PER_GRP + jj
                            sl = slice(tl * K, (tl + 1) * K)
                            # even row: PE tile (0, 0); odd: (64, 64)
                            nc.tensor.matmul(
                                out=ps[0:64, jj : jj + 1],
                                lhsT=cur[j][0:64, sl],
                                rhs=a_prev[0:64, jj : jj + 1],
                                start=True,
                                stop=True,
                            )
                            nc.tensor.matmul(
                                out=ps[64:128, jj : jj + 1],
                                lhsT=cur[j][64:128, sl],
                                rhs=a_prev[64:128, jj : jj + 1],
                                start=True,
                                stop=True,
                            )
                        nc.vector.tensor_copy(
                            a_bufs[g][t % 3][:],
                            ps[:, 0:PAIRS_PER_GRP],
                        )

            # ---- final state readout ----------------------------------
            for g in range(NGRP):
                nc.sync.dma_start(
                    out=afin[g][:], in_=a_bufs[g][(T - 1) % 3][:]
                )

    return nc


_NC_CACHE = None


def _get_nc():
    global _NC_CACHE
    if _NC_CACHE is None:
        _NC_CACHE = _build_nc()
        _NC_CACHE.finalize()
    return _NC_CACHE


def _make_in_maps(scores, targets, lengths):
    scores = np.asarray(scores, dtype=np.float32)
    targets = np.asarray(targets).astype(np.int64)
    lengths = np.asarray(lengths).astype(np.int64)

    # fold the per-step 2^-7 scale into the scores, then overwrite the
    # padded timesteps with the identity slab (exp == I exactly).
    shifted = scores - np.float32(LOG_C)
    pad_slab = np.full((K, K), PAD_OFFDIAG, dtype=np.float32)
    np.fill_diagonal(pad_slab, 0.0)
    for b in range(B):
        L = int(lengths[b])
        if L < T:
            shifted[b, L:] = pad_slab

    # a_0 = exp(sc'[b, 0, START, :]) per row (includes one 2^-7 factor)
    a0_all = np.exp(shifted[:, 0, START, :])  # [B, K] f64->f32
    a0_all = a0_all.astype(np.float32)

    in_maps = []
    for c in range(NCORES):
        sl = slice(c * BL, (c + 1) * BL)
        sh = shifted[sl]          # [BL, T, K, K]
        tg = targets[sl]          # [BL, T]
        ln = lengths[sl]          # [BL]

        # interleaved layout [b][blk][kf][tin][kto], bf16
        u = np.ascontiguousarray(
            sh.reshape(BL, NBLK, W, K, K).transpose(0, 1, 3, 2, 4)
        ).astype(BF16NP).reshape(BL, NBLK, K, W * K)

        # packed a0 per group: col jj = [row 2j state; row 2j+1 state]
        a0g = []
        for g in range(NGRP):
            arr = np.zeros((128, PAIRS_PER_GRP), dtype=BF16NP)
            for jj in range(PAIRS_PER_GRP):
                for h in range(2):
                    bl = 2 * (g * PAIRS_PER_GRP + jj) + h
                    arr[64 * h : 64 * h + 64, jj] = a0_all[
                        c * BL + bl
                    ].astype(BF16NP)
            a0g.append(arr)

        # gold gather element indices into the raw f32 scores shard
        b_idx = np.arange(BL)[:, None]
        t_idx = np.arange(T)[None, :]
        flat = (b_idx * T + t_idx) * (K * K) + tg  # [BL, T]
        valid = t_idx < ln[:, None]
        flat = np.where(valid, flat, np.int64(SENTINEL))
        gidx = flat.reshape(128, G).astype(np.int32)

        in_maps.append(
            {
                "u": u,
                "sc": np.ascontiguousarray(scores[sl]),
                "a00": a0g[0],
                "a01": a0g[1],
                "gidx": np.ascontiguousarray(gidx),
            }
        )
    return in_maps, lengths


def _combine(results, lengths):
    # a_fin carries L factors of 2^-7 (one from a_0, one per real step),
    # so fs_b = log(a_fin[END]) + L_b * LOG_C; gold is gathered from the
    # raw (unshifted) scores.
    all_scores = 0.0
    gold_total = 0.0
    for c in range(NCORES):
        gold_total += float(results[c]["goldv"][0, 0])
        afin = [results[c][f"afin{g}"].astype(np.float32) for g in range(NGRP)]
        for bl in range(BL):
            L = int(lengths[c * BL + bl])
            g = bl // (2 * PAIRS_PER_GRP)
            jj = (bl // 2) % PAIRS_PER_GRP
            h = bl % 2
            a_end = float(afin[g][64 * h + END, jj])
            all_scores += math.log(a_end) + L * LOG_C
    return np.float32((all_scores - gold_total) / B)


def kernel(scores, targets, lengths, trace=False):
    nc = _get_nc()
    in_maps, ln = _make_in_maps(scores, targets, lengths)
    res = run_bass_kernel_spmd(
        nc, in_maps, core_ids=list(range(NCORES)), trace=trace
    )
    out = _combine(res.results, ln)
    if trace:
        return out, res
    return out
